# revision 20
# baseline (speedup 1.0000x reference)
"""Multi-head attention kernel for Trainium2, 8 NeuronCores.

Sharding: 16 (batch, head) pairs -> 8 cores; core c handles batch c//4 and
heads {2*(c%4), 2*(c%4)+1}.  Megatron-style: each core gets only its heads'
rows of Wq/Wk/Wv and columns of Wo.  Outputs: per-head TRANSPOSED attention
[sk, sq] (host transposes back) and a partial output projection (host sums
the 4 cores of each batch and adds bo).
"""

import os
import numpy as np

HID = 512
HEADS = 8
HD = 64
B = 2
S = 2048
NCORES = 8

_PROGRAM_CACHE = {}
LAST_RESULT = None


def _build_program(s, mm_dtype_name="float32r"):
    import concourse.bass as bass
    import concourse.mybir as mybir
    import concourse.tile as tile
    from concourse import bacc

    f32 = mybir.dt.float32
    MMDT = getattr(mybir.dt, mm_dtype_name)
    AF = mybir.ActivationFunctionType
    ALU = mybir.AluOpType

    SB = s // 128            # number of 128-row blocks along s
    HALF = s // 2            # sq half width (pass-1 granularity)
    TW = min(512, HALF)      # matmul N tile width in pass 1
    NT = HALF // TW          # N tiles per half
    SBH = HALF // 128        # sq 128-blocks per half
    DC = HID // 128          # contraction chunks for projections
    PSW = min(1024, s)       # psum score region width (projections)
    VREG = min(512, s)       # V-projection psum region width

    nc = bacc.Bacc()

    xq_t = nc.declare_dram_parameter("xq_t", [HID, s], MMDT, isOutput=False)
    xk_t = nc.declare_dram_parameter("xk_t", [HID, s], MMDT, isOutput=False)
    xv_t = nc.declare_dram_parameter("xv_t", [HID, s], MMDT, isOutput=False)
    wq_t = nc.declare_dram_parameter("wq_t", [HID, 128], MMDT, isOutput=False)
    wk_t = nc.declare_dram_parameter("wk_t", [HID, 128], MMDT, isOutput=False)
    wv_t = nc.declare_dram_parameter("wv_t", [HID, 128], MMDT, isOutput=False)
    wo_t = nc.declare_dram_parameter("wo_t", [128, HID], MMDT, isOutput=False)
    bq_d = nc.declare_dram_parameter("bq", [128, 1], f32, isOutput=False)
    bk_d = nc.declare_dram_parameter("bk", [128, 1], f32, isOutput=False)
    bv_d = nc.declare_dram_parameter("bv", [1, 128], MMDT, isOutput=False)
    ones_c_d = nc.declare_dram_parameter("ones_c", [128, 1], MMDT, isOutput=False)
    ones_r_d = nc.declare_dram_parameter("ones_r", [1, 128], MMDT, isOutput=False)

    attn_t = nc.declare_dram_parameter("attn_t", [2, s, s], f32, isOutput=True)
    outp = nc.declare_dram_parameter("outp", [s, HID], f32, isOutput=True)

    recip_dram = nc.dram_tensor("recip_scratch", [2, s], f32)

    def mm(out, lhsT, rhs, **kw):
        nc.tensor.matmul(out, lhsT, rhs, **kw)

    with tile.TileContext(nc) as tc:
        with (
            tc.tile_pool(name="const", bufs=1) as const,
            tc.tile_pool(name="persist", bufs=1) as persist,
            tc.tile_pool(name="xin", bufs=2) as xin,
            tc.tile_pool(name="exp", bufs=SB + 1) as exppool,
            tc.tile_pool(name="attn", bufs=3) as apool,
            tc.tile_pool(name="rbc", bufs=2) as rbc,
            tc.tile_pool(name="dr", bufs=2) as drpool,
            tc.tile_pool(name="osum", bufs=3) as osum,
            tc.tile_pool(name="otmp", bufs=3) as otmp,
            tc.tile_pool(name="score", bufs=2, space="PSUM") as score,
            tc.tile_pool(name="acc", bufs=4, space="PSUM") as acc,
        ):
            # ---- constants -------------------------------------------------
            wq_sb = const.tile([128, DC, 128], MMDT, tag="wq")
            wk_sb = const.tile([128, DC, 128], MMDT, tag="wk")
            wv_sb = const.tile([128, DC, 128], MMDT, tag="wv")
            nc.sync.dma_start(wq_sb[:], wq_t.rearrange("(c p) m -> p c m", p=128))
            nc.sync.dma_start(wk_sb[:], wk_t.rearrange("(c p) m -> p c m", p=128))
            nc.sync.dma_start(wv_sb[:], wv_t.rearrange("(c p) m -> p c m", p=128))
            wo_sb = []
            for h in range(2):
                t = const.tile([64, HID], MMDT, tag=f"wo{h}", name=f"wo{h}")
                nc.sync.dma_start(t[:], wo_t[h * 64:(h + 1) * 64, :])
                wo_sb.append(t)
            bq_sb = const.tile([128, 1], f32, tag="bq")
            bk_sb = const.tile([128, 1], f32, tag="bk")
            bv_sb = const.tile([1, 128], MMDT, tag="bv")
            nc.sync.dma_start(bq_sb[:], bq_d[:])
            nc.sync.dma_start(bk_sb[:], bk_d[:])
            nc.sync.dma_start(bv_sb[:], bv_d[:])
            ones_sb = const.tile([128, 1], MMDT, tag="ones")
            ones_row = const.tile([1, 128], MMDT, tag="ones_row")
            nc.sync.dma_start(ones_sb[:], ones_c_d[:])
            nc.sync.dma_start(ones_row[:], ones_r_d[:])

            # ---- persistent activations -----------------------------------
            qht = persist.tile([128, s], MMDT, tag="qht")   # heads stacked on partitions
            kht = persist.tile([128, s], MMDT, tag="kht")
            vh = [persist.tile([128, SB, HD], MMDT, tag=f"vh{h}", name=f"vh{h}") for h in range(2)]
            xh = [persist.tile([64, s], MMDT, tag=f"xh{h}", name=f"xh{h}") for h in range(2)]
            rpp = [persist.tile([128, SB], f32, tag=f"rpp{h}", name=f"rpp{h}") for h in range(2)]

            # ---- Q/K projections: out QhT/KhT [j, s] ----------------------
            PT = min(512, PSW)
            NJ = PSW // PT
            for xdram, w_sb, b_sb, dst in (
                (xq_t, wq_sb, bq_sb, qht),
                (xk_t, wk_sb, bk_sb, kht),
            ):
                regions = [
                    score.tile([128, PSW], f32, tag="score", name="ps_proj")
                    for _r in range(s // PSW)
                ]
                for dc in range(DC):
                    xc = xin.tile([128, s], MMDT, tag="xc", name="xc")
                    nc.gpsimd.dma_start(xc[:], xdram[dc * 128:(dc + 1) * 128, :])
                    for r, ps in enumerate(regions):
                        for j2 in range(NJ):
                            mm(
                                ps[:, j2 * PT:(j2 + 1) * PT],
                                w_sb[:, dc, :],
                                xc[:, r * PSW + j2 * PT: r * PSW + (j2 + 1) * PT],
                                start=(dc == 0),
                                stop=(dc == DC - 1),
                            )
                for r, ps in enumerate(regions):
                    nc.scalar.add(dst[:, r * PSW:(r + 1) * PSW], ps[:], b_sb[:])

            # ---- V projection: natural layout [sk, j] ---------------------
            NBLK = VREG // 128
            vregions = [
                acc.tile([128, VREG], f32, tag="acc", name="ps_v")
                for _r in range(s // VREG)
            ]
            for dc in range(DC):
                xc = xin.tile([128, s], MMDT, tag="xc", name="xc")
                nc.gpsimd.dma_start(xc[:], xv_t[dc * 128:(dc + 1) * 128, :])
                for r, ps in enumerate(vregions):
                    for i in range(NBLK):
                        sblk = r * NBLK + i
                        mm(
                            ps[:, i * 128:(i + 1) * 128],
                            xc[:, sblk * 128:(sblk + 1) * 128],
                            wv_sb[:, dc, :],
                            start=(dc == 0 and i == 0),
                            stop=(dc == DC - 1 and i == NBLK - 1),
                        )
            for r, ps in enumerate(vregions):
                for i in range(NBLK):
                    sblk = r * NBLK + i
                    for h in range(2):
                        nc.vector.tensor_copy(
                            out=vh[h][:, sblk, :],
                            in_=ps[:, i * 128 + h * 64: i * 128 + h * 64 + 64],
                        )

            # ---- attention passes ----------------------------------------
            for h in range(2):
                hs = h * 64
                for v in range(2):
                    n_ps = [acc.tile([64, TW], f32, tag="acc", name=f"n_ps{_j}") for _j in range(NT)]
                    d_ps = [acc.tile([1, TW], f32, tag="acc", name=f"d_ps{_j}") for _j in range(NT)]
                    ets = []
                    for blk in range(SB):
                        sT = score.tile([128, HALF], f32, tag="score", name="sT")
                        for j in range(NT):
                            mm(
                                sT[:, j * TW:(j + 1) * TW],
                                kht[hs:hs + 64, blk * 128:(blk + 1) * 128],
                                qht[hs:hs + 64, v * HALF + j * TW: v * HALF + (j + 1) * TW],
                                start=True,
                                stop=True,
                            )
                        eT = exppool.tile([128, HALF], MMDT, tag="exp", name="eT")
                        nc.scalar.activation(eT[:], sT[:], AF.Exp, scale=0.125)
                        for j in range(NT):
                            mm(
                                n_ps[j][:],
                                vh[h][:, blk, :],
                                eT[:, j * TW:(j + 1) * TW],
                                start=(blk == 0),
                                stop=False,
                            )
                            mm(
                                d_ps[j][:],
                                ones_sb[:],
                                eT[:, j * TW:(j + 1) * TW],
                                start=(blk == 0),
                                stop=(blk == SB - 1),
                            )
                        ets.append(eT)

                    den_t = drpool.tile([1, HALF], f32, tag="den", name="den_t")
                    rec_t = drpool.tile([1, HALF], f32, tag="rec", name="rec_t")
                    den_r = drpool.tile([1, HALF], MMDT, tag="den_r", name="den_r")
                    rec_r = drpool.tile([1, HALF], MMDT, tag="rec_r", name="rec_r")
                    for j in range(NT):
                        jsl = slice(j * TW, (j + 1) * TW)
                        dsl = slice(v * HALF + j * TW, v * HALF + (j + 1) * TW)
                        nc.vector.tensor_copy(out=den_t[0:1, jsl], in_=d_ps[j][:])
                        nc.vector.tensor_copy(out=den_r[0:1, jsl], in_=d_ps[j][:])
                        # fold V bias: xh += bv ⊗ denom
                        mm(
                            n_ps[j][:],
                            bv_sb[0:1, hs:hs + 64],
                            den_r[0:1, jsl],
                            start=False,
                            stop=True,
                        )
                        nc.vector.tensor_copy(out=xh[h][:, dsl], in_=n_ps[j][:])
                    vsl = slice(v * HALF, (v + 1) * HALF)
                    nc.vector.reciprocal(rec_t[:], den_t[:])
                    nc.vector.tensor_copy(out=rec_r[:], in_=rec_t[:])
                    nc.gpsimd.dma_start(recip_dram[h, vsl], rec_t[:])
                    with nc.allow_non_contiguous_dma(reason="tiny recip transpose"):
                        nc.gpsimd.dma_start(
                            rpp[h][:, v * SBH:(v + 1) * SBH],
                            recip_dram[h, vsl].rearrange("(b p) -> p b", p=128),
                        )
                    rb = rbc.tile([128, HALF], f32, tag="rbc", name="rb")
                    rb_ps = score.tile([128, HALF], f32, tag="score", name="rb_ps")
                    for j in range(NT):
                        mm(
                            rb_ps[:, j * TW:(j + 1) * TW],
                            ones_row[:],
                            rec_r[0:1, j * TW:(j + 1) * TW],
                            start=True,
                            stop=True,
                        )
                    nc.vector.tensor_copy(out=rb[:], in_=rb_ps[:])
                    for blk in range(SB):
                        aT = apool.tile([128, HALF], f32, tag="attn", name="aT")
                        nc.vector.tensor_tensor(aT[:], ets[blk][:], rb[:], ALU.mult)
                        nc.sync.dma_start(
                            attn_t[h, blk * 128:(blk + 1) * 128, vsl], aT[:]
                        )

            # ---- output projection (both heads, summed on-chip) ----------
            for blk in range(SB):
                osb = osum.tile([128, HID], f32, tag="osum", name="osb")
                for h in range(2):
                    wps = acc.tile([128, HID], f32, tag="acc", name="wps")
                    mm(
                        wps[:],
                        xh[h][:, blk * 128:(blk + 1) * 128],
                        wo_sb[h][:],
                        start=True,
                        stop=True,
                    )
                    if h == 0:
                        nc.scalar.activation(
                            osb[:], wps[:], AF.Copy, scale=rpp[0][:, blk:blk + 1]
                        )
                    else:
                        tmp = otmp.tile([128, HID], f32, tag="otmp", name="otmp")
                        nc.scalar.activation(
                            tmp[:], wps[:], AF.Copy, scale=rpp[1][:, blk:blk + 1]
                        )
                        nc.vector.tensor_tensor(osb[:], osb[:], tmp[:], ALU.add)
                        nc.sync.dma_start(outp[blk * 128:(blk + 1) * 128, :], osb[:])

    nc.finalize()
    return nc


def _get_program(s=S, mm_dtype_name=None):
    if mm_dtype_name is None:
        mm_dtype_name = os.environ.get("KERNEL_MM_DTYPE", "float32r")
    key = (s, mm_dtype_name)
    if key not in _PROGRAM_CACHE:
        _PROGRAM_CACHE[key] = _build_program(s, mm_dtype_name)
    return _PROGRAM_CACHE[key]


def _make_in_maps(q, k, v, Wq, bq, Wk, bk, Wv, bv, Wo):
    in_maps = []
    for c in range(NCORES):
        b = c // 4
        h0 = 2 * (c % 4)
        r0 = h0 * HD
        in_maps.append(
            {
                "xq_t": np.ascontiguousarray(q[b].T),
                "xk_t": np.ascontiguousarray(k[b].T),
                "xv_t": np.ascontiguousarray(v[b].T),
                "wq_t": np.ascontiguousarray(Wq[r0:r0 + 128, :].T),
                "wk_t": np.ascontiguousarray(Wk[r0:r0 + 128, :].T),
                "wv_t": np.ascontiguousarray(Wv[r0:r0 + 128, :].T),
                "wo_t": np.ascontiguousarray(Wo[:, r0:r0 + 128].T),
                "bq": np.ascontiguousarray(bq[r0:r0 + 128, None]),
                "bk": np.ascontiguousarray(bk[r0:r0 + 128, None]),
                "bv": np.ascontiguousarray(bv[None, r0:r0 + 128]),
                "ones_c": np.ones((128, 1), np.float32),
                "ones_r": np.ones((1, 128), np.float32),
            }
        )
    return in_maps


def _assemble(results, bo):
    out = np.zeros([B, S, HID], np.float32)
    attn = np.empty([B, HEADS, S, S], np.float32)
    for c in range(NCORES):
        b = c // 4
        h0 = 2 * (c % 4)
        r = results[c]
        out[b] += r["outp"]
        at = r["attn_t"]
        attn[b, h0] = at[0].T
        attn[b, h0 + 1] = at[1].T
    out += bo[None, None, :].astype(np.float32)
    return out, attn


def _install_ntff_hook():
    """The agent image's antenv lacks axon_hooks; recreate it so
    run_bass_kernel_spmd(trace=True) can profile via the axon .so."""
    import sys
    import types

    try:
        from antenv.axon_hooks import get_axon_ntff_profile_hook  # noqa: F401

        return
    except ImportError:
        pass
    from trn_agent_boot.trn_boot import _ntff_profile_via_ctypes

    hook = _ntff_profile_via_ctypes("/opt/axon/libaxon_pjrt.so")
    m = types.ModuleType("antenv.axon_hooks")
    m.get_axon_ntff_profile_hook = lambda: hook
    m.set_axon_ntff_profile_hook = lambda h: None
    sys.modules["antenv.axon_hooks"] = m


def _fallback(q, k, v, mask, Wq, bq, Wk, bk, Wv, bv, Wo, bo):
    """Pure-numpy reference path (used only if mask is not all-ones)."""
    qh = (q @ Wq.T + bq).reshape(B, S, HEADS, HD).transpose(0, 2, 1, 3)
    kh = (k @ Wk.T + bk).reshape(B, S, HEADS, HD).transpose(0, 2, 1, 3)
    vh = (v @ Wv.T + bv).reshape(B, S, HEADS, HD).transpose(0, 2, 1, 3)
    scale = 1.0 / np.sqrt(np.float32(HD))
    attn = np.empty([B, HEADS, S, S], np.float32)
    x = np.empty([B, HEADS, S, HD], np.float32)
    for b in range(B):
        for h in range(HEADS):
            sc = (qh[b, h] @ kh[b, h].T) * scale
            sc = np.where(mask[b, 0] == 0, np.float32(-1e10), sc)
            sc = sc - sc.max(axis=-1, keepdims=True)
            e = np.exp(sc)
            a = e / e.sum(axis=-1, keepdims=True)
            attn[b, h] = a
            x[b, h] = a @ vh[b, h]
    xo = x.transpose(0, 2, 1, 3).reshape(B, S, HID)
    out = xo @ Wo.T + bo
    return out.astype(np.float32), attn


def kernel(**inputs):
    q = np.asarray(inputs["q"], np.float32)
    k = np.asarray(inputs["k"], np.float32)
    v = np.asarray(inputs["v"], np.float32)
    mask = np.asarray(inputs["mask"])
    Wq = np.asarray(inputs["Wq"], np.float32)
    bq = np.asarray(inputs["bq"], np.float32)
    Wk = np.asarray(inputs["Wk"], np.float32)
    bk = np.asarray(inputs["bk"], np.float32)
    Wv = np.asarray(inputs["Wv"], np.float32)
    bv = np.asarray(inputs["bv"], np.float32)
    Wo = np.asarray(inputs["Wo"], np.float32)
    bo = np.asarray(inputs["bo"], np.float32)

    if not np.all(mask == 1):
        return _fallback(q, k, v, mask, Wq, bq, Wk, bk, Wv, bv, Wo, bo)

    from concourse import bass_utils

    nc = _get_program()
    in_maps = _make_in_maps(q, k, v, Wq, bq, Wk, bk, Wv, bv, Wo)
    trace = os.environ.get("KERNEL_TRACE") == "1"
    kw = {}
    if trace:
        _install_ntff_hook()
        bass_utils.upload_artifacts = lambda tmpdir: f"local:{tmpdir}"
        tdir = os.environ.get("KERNEL_TRACE_DIR")
        if tdir:
            import time as _time

            tdir = os.path.join(tdir, f"run_{int(_time.time() * 1000)}")
            os.makedirs(tdir, exist_ok=True)
            kw["tmpdir"] = tdir
    res = bass_utils.run_bass_kernel_spmd(
        nc, in_maps, list(range(NCORES)), trace=trace, **kw
    )
    global LAST_RESULT
    LAST_RESULT = res
    return _assemble(res.results, bo)
